# revision 14
# baseline (speedup 1.0000x reference)
"""Trainium2 Bass kernel for ChannelAwareAttentionModule.

Reference computation (per sample b, x: (256, 4096) = (C, H*W)):
    g     = relu(BN(Wg  @ x))                  (128, 4096)
    theta = relu(BN(Wth @ x))                  (128, 4096)
    phi   = relu(BN(Wph @ x))                  (128, 4096)
    f     = softmax(theta @ phi.T, axis=-1)    (128, 128)
    y     = f @ g                              (128, 4096)
    z     = y.T.reshape(128, 4096)             (torch permute+view scramble)
    out   = BN(Ww @ z) + x                     (256, 4096)

Sharding: pure data-parallel, 2 samples per core on 8 cores.

Kernel-level transformations:
  * BN folded into conv weights/biases on the host; all operands cast to
    fp16 on the host (fp32 accumulation in PSUM throughout).
  * Final-proj bias folded into x (x_adj = x + bw), projection biases
    compensated; the residual (+ x_adj) is added during PSUM evacuation
    via scalar_tensor_tensor on DVE (or via an identity matmul when the
    ACT engine evacuates).
  * theta/phi are computed directly in n-transposed layout (needed by the
    scores matmul) and kept INTERLEAVED (th|ph per 256-col block) so each
    (128,1024) PSUM tile evacuates with a single contiguous 1024-wide op.
  * The permute+view scramble z[c', q*128+r] = y[r, 32c'+q] is realized by
    a stride-32 lhsT access on g (g stored permuted at evac time), so no
    transpose of y is ever materialized.
  * All PSUM evacuations are single 1024-wide ops (ACT ~1.15us / DVE
    ~1.24us each) load-balanced across the two PSUM-reading engines.
  * One shared 3-deep (128,1024) PSUM pool decouples PE from evacuation.
  * A dummy activation right after the preamble forces the ACT table load
    early so ACT can evacuate from ~8us.
  * Input DMA issues are split across the two HWDGE queues (Sync: consts +
    sample 0, ACT: bg + sample 1) - each DMA_DIRECT2D costs ~640ns of
    issue time on its queue, which otherwise serializes input delivery.
  * A burst of dummy matmuls on zeroed scratch runs during the initial
    DMA window so the PE HAM clock-gate un-throttles (1.2 -> 2.4 GHz)
    before real matmuls start.
  * Per-sample software pipeline: sample 0's attention+output overlaps
    sample 1's projections; output DMAs overlap compute.
"""

from contextlib import ExitStack

import numpy as np

import concourse.bacc as bacc
import concourse.mybir as mybir
from concourse import tile
from concourse.bass_utils import run_bass_kernel_spmd

F32 = mybir.dt.float32
F16 = mybir.dt.float16
AF = mybir.ActivationFunctionType
ALU = mybir.AluOpType
AX = mybir.AxisListType

NCORES = 8
B, C, CI, N = 16, 256, 128, 4096
BPC = B // NCORES  # samples per core
NQ = N // 128  # 32 column blocks of 128
EPS = 1e-5
NWARM = 8  # HAM warm-up matmuls (N=512 each, cold ~610ns)

# const tensor column layout: [bt4(640) | wtp(512) | wr(640)]
CST_BT4 = 0
CST_WTP = 640
CST_WR = 1152
CST_COLS = 1792


def _build_nc():
    nc = bacc.Bacc("TRN2", target_bir_lowering=False, debug=False, num_devices=NCORES)

    x_d = nc.dram_tensor("x", [BPC, C, N], F16, kind="ExternalInput")
    cst_d = nc.dram_tensor("cst", [128, CST_COLS], F16, kind="ExternalInput")
    bg_d = nc.dram_tensor("bg", [CI, 1], F32, kind="ExternalInput")
    out_d = nc.dram_tensor("out", [BPC, C, N], F16, kind="ExternalOutput")

    with tile.TileContext(nc) as tc, ExitStack() as ctx:
        wpool = ctx.enter_context(tc.tile_pool(name="wts", bufs=1))
        xpool = ctx.enter_context(tc.tile_pool(name="xp", bufs=BPC))
        tppool = ctx.enter_context(tc.tile_pool(name="tpp", bufs=BPC))
        gpool = ctx.enter_context(tc.tile_pool(name="gp", bufs=BPC))
        zpool = ctx.enter_context(tc.tile_pool(name="zp", bufs=BPC))
        smpool = ctx.enter_context(tc.tile_pool(name="sm", bufs=BPC))
        opool = ctx.enter_context(tc.tile_pool(name="ost", bufs=4))
        ps_big = ctx.enter_context(tc.tile_pool(name="ps_big", bufs=3, space="PSUM"))
        ps_sml = ctx.enter_context(tc.tile_pool(name="ps_sml", bufs=2, space="PSUM"))

        # --- constants + scratch ---
        cst = wpool.tile([128, CST_COLS], F16, tag="cst")
        bg_sb = wpool.tile([CI, 1], F32, tag="bg")
        scr = wpool.tile([128, 512], F16, tag="scr")

        # warm-up: dummy matmuls during the input-DMA window so the HAM
        # clock gate reaches 8/8 before the first real matmul; cold N=512
        # matmuls (~610ns each) bridge until the first x chunks land.
        nc.vector.memset(scr[:, :], 0.0)
        ps_w = ps_big.tile([128, 1024], F32, tag="big", name="ps_warm")
        for _ in range(NWARM):
            nc.tensor.matmul(
                ps_w[:, 0:512], scr[:, 0:128], scr[:, :],
                start=True, stop=True, skip_group_check=True,
            )

        # --- input DMAs ---
        # x chunks alternate between the two HWDGE queues (Sync and ACT) so
        # the ~640ns-per-issue serialization doesn't throttle the feed.
        # Sample 1's chunks are issued mid-program from the ACT queue.
        xs = {}
        for b in range(BPC):
            xs[b] = xpool.tile([128, 2, N], F16, tag="x", name=f"x_{b}")

        def x_dma(eng, b, j):
            # 512-col chunk j of sample b
            eng.dma_start(
                xs[b][:, :, 512 * j : 512 * (j + 1)],
                x_d[b, :, 512 * j : 512 * (j + 1)].rearrange("(k p) n -> p k n", k=2),
            )

        nc.sync.dma_start(cst[:, :], cst_d[:])
        nc.sync.dma_start(xs[0][:, 0, 0:512], x_d[0, 0:128, 0:512])
        nc.scalar.dma_start(xs[0][:, 1, 0:512], x_d[0, 128:256, 0:512])
        for j in range(1, 8):
            x_dma(nc.sync if j % 2 else nc.scalar, 0, j)
        nc.scalar.dma_start(bg_sb[:, 0:1], bg_d[:])
        # sample-1 chunks queue on Sync behind sample 0's odd chunks; they
        # transfer ~11-19us, well before sample-1 compute needs them
        for m in range(4):
            nc.sync.dma_start(
                xs[1][:, :, 1024 * m : 1024 * (m + 1)],
                x_d[1, :, 1024 * m : 1024 * (m + 1)].rearrange("(k p) n -> p k n", k=2),
            )
        busy = {"act": 5 * 650.0, "dve": 0.0}

        wtp = cst[:, CST_WTP : CST_WTP + 512]
        w_g = cst[:, CST_WR : CST_WR + 256]
        w_w = cst[:, CST_WR + 256 : CST_WR + 512]
        idf = cst[:, CST_WR + 512 : CST_WR + 640]
        # bias row + ones row replicated at partitions 0/32/64/96 so up to 4
        # rank-1 bias matmuls can run in disjoint PE row groups concurrently
        btp = [cst[32 * r : 32 * r + 1, CST_BT4 : CST_BT4 + 512] for r in range(4)]
        onesr = [
            cst[32 * r : 32 * r + 1, CST_BT4 + 512 : CST_BT4 + 640] for r in range(4)
        ]

        # --- evacuation-engine load balancer (only ACT/DVE read PSUM) ---
        def cost(e, nlane):
            # ns per op with nlane elems per partition lane
            if e == "act":
                return (nlane + 352) / 1.2
            return nlane * 1.042 + 170.0

        def pick(nlane, allowed):
            e = min(allowed, key=lambda k: busy[k] + cost(k, nlane))
            busy[e] += cost(e, nlane)
            return e

        def evac_relu(dst, src, nlane, allowed=("act", "dve")):
            e = pick(nlane, allowed)
            if e == "act":
                nc.scalar.activation(dst, src, AF.Relu)
            else:
                nc.vector.tensor_scalar(dst, src, 0.0, None, ALU.max)

        def evac_relu_bias(dst, src, bias, nlane, allowed=("act", "dve")):
            e = pick(nlane, allowed)
            if e == "act":
                nc.scalar.activation(dst, src, AF.Relu, bias=bias)
            else:
                nc.vector.tensor_scalar(dst, src, bias, 0.0, ALU.add, ALU.max)

        def evac_copy(dst, src, nlane, allowed=("act", "dve")):
            e = pick(nlane, allowed)
            if e == "act":
                nc.scalar.copy(dst, src)
            else:
                nc.vector.tensor_copy(dst, src)

        tp_sb, g_sb, z_sb, f_soft, fT_sb = {}, {}, {}, {}, {}
        for b in range(BPC):
            tp_sb[b] = tppool.tile([128, 2 * N], F16, tag="tp", name=f"tp_{b}")
            g_sb[b] = gpool.tile([128, N], F16, tag="g", name=f"g_{b}")
            z_sb[b] = zpool.tile([128, N], F16, tag="z", name=f"z_{b}")

        def _thph_bias(pt, rbase):
            # 2 rank-1 bias matmuls (one per 512-col bank) in disjoint PE
            # row groups -> run concurrently with each other (and with the
            # other tile's pair when quad-packed).
            for half in range(2):
                r = rbase + half
                nc.tensor.matmul(
                    pt[:, 512 * half : 512 * (half + 1)],
                    onesr[r], btp[r],
                    start=True, stop=False, skip_group_check=True,
                    tile_position=(32 * r, 0),
                )

        def _thph_body(b, pt, t, stop):
            for h in range(4):
                q = 4 * t + h
                for k in range(2):
                    nc.tensor.matmul(
                        pt[:, 256 * h : 256 * (h + 1)],
                        xs[b][:, k, 128 * q : 128 * (q + 1)],
                        wtp[:, 256 * k : 256 * (k + 1)],
                        start=False,
                        stop=(stop and h == 3 and k == 1),
                        skip_group_check=True,
                    )

        def _thph_evac(b, pt, t, allowed=("act", "dve")):
            # single contiguous 1024-wide evac: PSUM tile layout is already
            # the interleaved (th|ph per 256-block) SBUF layout
            evac_relu(
                tp_sb[b][:, 1024 * t : 1024 * (t + 1)], pt[:, :], 1024, allowed
            )

        def proj_thph(b, trange, allowed=("act", "dve")):
            for t in trange:
                pt = ps_big.tile([128, 1024], F32, tag="big", name=f"pt_{b}_{t}")
                _thph_bias(pt, 0)
                _thph_body(b, pt, t, True)
                _thph_evac(b, pt, t, allowed)

        def proj_thph_quad(b, tpairs, allowed=("act", "dve")):
            # process t in pairs: all 4 bias matmuls issue back-to-back into
            # 4 distinct PE row groups -> ~4x faster than serial rank-1 MMs
            for t0 in tpairs:
                pa = ps_big.tile([128, 1024], F32, tag="big", name=f"pt_{b}_{t0}")
                pb = ps_big.tile([128, 1024], F32, tag="big", name=f"pt_{b}_{t0+1}")
                _thph_bias(pa, 0)
                _thph_bias(pb, 2)
                _thph_body(b, pa, t0, True)
                _thph_evac(b, pa, t0, allowed)
                _thph_body(b, pb, t0 + 1, True)
                _thph_evac(b, pb, t0 + 1, allowed)

        def proj_g(b, trange=range(4), allowed=("act", "dve")):
            # evacuate g directly into the permuted layout
            # gP[d, 128q + c] = g[d, 32c + q]  (c = 16j + a, n = 32a + q)
            # single 4D-AP evac per (128,1024) PSUM tile
            gPv = g_sb[b][:, :].rearrange("p (r t h a) -> p r t h a", r=NQ, t=4, h=2)
            for t in trange:
                pg = ps_big.tile([128, 1024], F32, tag="big", name=f"pg_{b}_{t}")
                for half in range(2):
                    j = 2 * t + half
                    for k in range(2):
                        nc.tensor.matmul(
                            pg[:, 512 * half : 512 * (half + 1)],
                            w_g[:, 128 * k : 128 * (k + 1)],
                            xs[b][:, k, 512 * j : 512 * (j + 1)],
                            start=(k == 0),
                            stop=(k == 1),
                            skip_group_check=True,
                        )
                src = pg[:, :].rearrange("p (h a r) -> p r h a", r=NQ, h=2)
                evac_relu_bias(
                    gPv[:, :, t, :, :], src, bg_sb[:, 0:1], 1024, allowed
                )

        def scores_part(b, qlo, qhi):
            # incremental slice of the (128,128) scores accumulation; the
            # partials interleave with projection matmuls so the PE stays
            # busy while x chunks land
            if qlo == 0:
                f_soft[b] = ps_sml.tile([128, 128], F32, tag="sml", name=f"ps_s_{b}")
            ps_s = f_soft[b]
            for q in range(qlo, qhi):
                nc.tensor.matmul(
                    ps_s[:, :],
                    tp_sb[b][:, 256 * q : 256 * q + 128],
                    tp_sb[b][:, 256 * q + 128 : 256 * (q + 1)],
                    start=(q == 0),
                    stop=(q == NQ - 1),
                    skip_group_check=True,
                )

        def softmax_ops(b):
            ps_s = f_soft[b]
            negmax = smpool.tile([128, 1], F32, tag="negmax", name=f"negmax_{b}")
            nc.vector.reduce_max(negmax[:, :], ps_s[:, :], axis=AX.X, negate=True)
            e_sb = smpool.tile([128, 128], F16, tag="e_sb", name=f"e_sb_{b}")
            sumex = smpool.tile([128, 1], F32, tag="sumex", name=f"sumex_{b}")
            nc.scalar.activation(
                e_sb[:, :], ps_s[:, :], AF.Exp, bias=negmax[:, :], accum_out=sumex[:, :]
            )
            rs = smpool.tile([128, 1], F32, tag="rs", name=f"rs_{b}")
            nc.vector.reciprocal(rs[:, :], sumex[:, :])
            f_sb = smpool.tile([128, 128], F16, tag="f_sb", name=f"f_sb_{b}")
            nc.scalar.activation(f_sb[:, :], e_sb[:, :], AF.Copy, scale=rs[:, :])
            f_soft[b] = f_sb
            busy["act"] += cost("act", 128) + cost("act", 128)
            busy["dve"] += 300.0

        def transpose_f(b):
            ps_t = ps_sml.tile([128, 128], F16, tag="sml", name=f"ps_t_{b}")
            nc.tensor.transpose(ps_t[:, :], f_soft[b][:, :], idf[:, :])
            fT = smpool.tile([128, 128], F16, tag="fT", name=f"fT_{b}")
            nc.vector.tensor_copy(fT[:, :], ps_t[:, :])
            busy["dve"] += cost("dve", 128)
            fT_sb[b] = fT

        def zblk(b, t):
            # z block t: 8 permuted-g matmuls + one 1024-wide PSUM->SBUF copy
            pz = ps_big.tile([128, 1024], F32, tag="big", name=f"pz_{b}_{t}")
            for tq in range(8):
                q = 8 * t + tq
                nc.tensor.matmul(
                    pz[:, 128 * tq : 128 * (tq + 1)],
                    g_sb[b][:, 128 * q : 128 * (q + 1)],
                    fT_sb[b][:, :],
                    start=True,
                    stop=True,
                    skip_group_check=True,
                )
            evac_copy(z_sb[b][:, 1024 * t : 1024 * (t + 1)], pz[:, :], 1024)

        otb = {}

        def fin(b, jj, split_out=False, allowed=("act", "dve")):
            # final projection + residual for n-chunk jj (512 cols, both
            # 128-row output halves) -> one (128,1024) PSUM tile, single
            # evac op.  Output staged in pairs (jj, jj+1) per DMA.
            z = z_sb[b]
            t, p = jj // 2, jj % 2
            if p == 0:
                otb[(b, t)] = opool.tile(
                    [128, 2, 1024], F16, tag="ost", name=f"ot_{b}_{t}"
                )
            ob = otb[(b, t)]
            pf = ps_big.tile([128, 1024], F32, tag="big", name=f"pf_{b}_{jj}")
            e = pick(1024, allowed)
            for h in range(2):
                if e == "act":
                    # residual via identity matmul, ACT copy out
                    nc.tensor.matmul(
                        pf[:, 512 * h : 512 * (h + 1)],
                        idf, xs[b][:, h, 512 * jj : 512 * (jj + 1)],
                        start=True, stop=False, skip_group_check=True,
                    )
                nc.tensor.matmul(
                    pf[:, 512 * h : 512 * (h + 1)],
                    w_w[:, 128 * h : 128 * (h + 1)],
                    z[:, 512 * jj : 512 * (jj + 1)],
                    start=(e != "act"), stop=True, skip_group_check=True,
                )
            ot = ob[:, :, 512 * p : 512 * (p + 1)]
            pfv = pf[:, :].rearrange("p (h n) -> p h n", h=2)
            if e == "act":
                nc.scalar.copy(ot, pfv)
            else:
                # residual fused into the DVE evacuation
                nc.vector.scalar_tensor_tensor(
                    ot, pfv, 0.0,
                    xs[b][:, :, 512 * jj : 512 * (jj + 1)],
                    ALU.add, ALU.add,
                )
            if split_out:
                nc.sync.dma_start(
                    out_d[
                        b, :, 1024 * t + 512 * p : 1024 * t + 512 * (p + 1)
                    ].rearrange("(h p) n -> p h n", h=2),
                    ot,
                )
            elif p == 1:
                nc.sync.dma_start(
                    out_d[b, :, 1024 * t : 1024 * (t + 1)].rearrange(
                        "(h p) n -> p h n", h=2
                    ),
                    ob[:, :, :],
                )

        # --- software pipeline over the 2 samples ---
        # Loop 1 (sample 0, DMA-feed-limited window): consume each landed
        # 1024-col x chunk completely - 2 thph tiles + 1 g tile + the
        # previous chunk's scores partials - so the PE never idles long
        # enough for the HAM clock gate to re-throttle.  The first three
        # thph evacs go to DVE only (ACT is still busy with DMA issues and
        # its activation-table load).
        for c in range(4):
            proj_thph_quad(
                0, (2 * c,), allowed=("dve",) if c == 0 else ("act", "dve")
            )
            proj_g(0, (c,))
            if c > 0:
                scores_part(0, 8 * (c - 1), 8 * c)
        scores_part(0, 24, 32)
        softmax_ops(0)
        # Loop 2 (sample 1 projections + sample 0 attention/output): the
        # x1 feed window gets sample 0's z/fin PE work interleaved so both
        # PE and the evacuation engines see a smooth load.
        proj_thph(1, (0,))
        transpose_f(0)
        proj_thph(1, (1,))
        for c in range(4):
            if c > 0:
                proj_thph_quad(1, (2 * c,))
            if c < 3:
                zblk(0, c)
            proj_g(1, (c,))
            if c > 0:
                scores_part(1, 8 * (c - 1), 8 * c)
            if c < 3:
                fin(0, 2 * c)
                fin(0, 2 * c + 1)
        scores_part(1, 24, 32)
        # sample-0's last attention block covers sample-1's softmax latency
        zblk(0, 3)
        softmax_ops(1)
        fin(0, 6)
        transpose_f(1)
        fin(0, 7)
        # tail: z blocks run one step ahead of their finals so fin matmuls
        # never wait on the immediately-preceding z evacuation
        zblk(1, 0)
        zblk(1, 1)
        fin(1, 0)
        fin(1, 1)
        zblk(1, 2)
        fin(1, 2)
        fin(1, 3)
        zblk(1, 3)
        fin(1, 4)
        fin(1, 5)
        fin(1, 6, split_out=True)
        fin(1, 7, split_out=True)

    nc.compile()
    return nc


_CACHE = {}


def _prepare(inputs):
    """Fold BN into weights/biases and build per-core input maps."""

    def fold(w, bias, gamma, beta, mean, var):
        inv = gamma / np.sqrt(var + EPS)
        return (w * inv[:, None]).astype(np.float32), (
            beta + (bias - mean) * inv
        ).astype(np.float32)

    Wg, bg = fold(
        inputs["g_w"], inputs["g_b"], inputs["g_gamma"], inputs["g_beta"],
        inputs["g_mean"], inputs["g_var"],
    )
    Wth, bth = fold(
        inputs["th_w"], inputs["th_b"], inputs["th_gamma"], inputs["th_beta"],
        inputs["th_mean"], inputs["th_var"],
    )
    Wph, bph = fold(
        inputs["ph_w"], inputs["ph_b"], inputs["ph_gamma"], inputs["ph_beta"],
        inputs["ph_mean"], inputs["ph_var"],
    )
    Ww, bw = fold(
        inputs["w_w"], inputs["w_b"], inputs["w_gamma"], inputs["w_beta"],
        inputs["w_mean"], inputs["w_var"],
    )

    # x_adj = x + bw (per out-channel); compensate projection biases.
    x = np.asarray(inputs["x"], dtype=np.float32).reshape(B, C, N)
    x_adj = (x + bw[None, :, None]).astype(np.float16)
    bg_a = bg - Wg @ bw
    bth_a = bth - Wth @ bw
    bph_a = bph - Wph @ bw

    WgT = np.ascontiguousarray(Wg.T)  # (256, 128)
    wg_host = np.concatenate([WgT[0:128], WgT[128:256]], axis=1)  # (128, 256)
    WtpT = np.concatenate([Wth.T, Wph.T], axis=1)  # (256, 256)
    wtp_host = np.concatenate([WtpT[0:128], WtpT[128:256]], axis=1)  # (128, 512)
    btp_host = np.concatenate([bth_a, bph_a, bth_a, bph_a])  # (512,)
    ww_host = np.ascontiguousarray(Ww.T)  # (128, 256)

    bt4 = np.zeros((128, 640), dtype=np.float16)
    for r in (0, 32, 64, 96):
        bt4[r, 0:512] = btp_host
        bt4[r, 512:640] = 1.0
    wr = np.zeros((128, 640), dtype=np.float16)
    wr[:, 0:256] = wg_host
    wr[:, 256:512] = ww_host
    wr[:, 512:640] = np.eye(128, dtype=np.float16)
    cst = np.concatenate([bt4, wtp_host.astype(np.float16), wr], axis=1)
    consts = {
        "cst": np.ascontiguousarray(cst, dtype=np.float16),
        "bg": np.ascontiguousarray(bg_a.reshape(CI, 1), dtype=np.float32),
    }
    in_maps = []
    for i in range(NCORES):
        m = dict(consts)
        m["x"] = np.ascontiguousarray(x_adj[BPC * i : BPC * (i + 1)])
        in_maps.append(m)
    return in_maps


def _get_nc():
    if "nc" not in _CACHE:
        _CACHE["nc"] = _build_nc()
    return _CACHE["nc"]


def run(inputs, **kw):
    """Run on hardware; returns (full_output, BassKernelResults)."""
    nc = _get_nc()
    in_maps = _prepare(inputs)
    res = run_bass_kernel_spmd(nc, in_maps, list(range(NCORES)), **kw)
    out = np.concatenate(
        [
            np.asarray(res.results[i]["out"], dtype=np.float32).reshape(BPC, C, 64, 64)
            for i in range(NCORES)
        ],
        axis=0,
    )
    return np.ascontiguousarray(out), res


def kernel(**inputs):
    out, _ = run(inputs)
    return out


# revision 24
# speedup vs baseline: 1.1274x; 1.1274x over previous
"""Trainium2 Bass kernel for ChannelAwareAttentionModule.

Reference computation (per sample b, x: (256, 4096) = (C, H*W)):
    g     = relu(BN(Wg  @ x))                  (128, 4096)
    theta = relu(BN(Wth @ x))                  (128, 4096)
    phi   = relu(BN(Wph @ x))                  (128, 4096)
    f     = softmax(theta @ phi.T, axis=-1)    (128, 128)
    y     = f @ g                              (128, 4096)
    z     = y.T.reshape(128, 4096)             (torch permute+view scramble)
    out   = BN(Ww @ z) + x                     (256, 4096)

Sharding: pure data-parallel, 2 samples per core on 8 cores.

Kernel-level transformations:
  * BN folded into conv weights/biases on the host; all operands cast to
    fp16 on the host (fp32 accumulation in PSUM throughout).
  * Final-proj bias folded into x (x_adj = x + bw), projection biases
    compensated; the residual (+ x_adj) is added during PSUM evacuation
    via scalar_tensor_tensor on DVE (or via an identity matmul when the
    ACT engine evacuates).
  * theta/phi are computed directly in n-transposed layout (needed by the
    scores matmul) and kept INTERLEAVED (th|ph per 256-col block) so each
    (128,1024) PSUM tile evacuates with a single contiguous 1024-wide op.
  * The permute+view scramble z[c', q*128+r] = y[r, 32c'+q] is realized by
    a stride-32 lhsT access on g (g stored permuted at evac time), so no
    transpose of y is ever materialized.
  * All PSUM evacuations are single 1024-wide ops (ACT ~1.15us / DVE
    ~1.24us each) load-balanced across the two PSUM-reading engines.
  * One shared 3-deep (128,1024) PSUM pool decouples PE from evacuation.
  * A dummy activation right after the preamble forces the ACT table load
    early so ACT can evacuate from ~8us.
  * Input DMA issues are split across the two HWDGE queues (Sync: consts +
    sample 0, ACT: bg + sample 1) - each DMA_DIRECT2D costs ~640ns of
    issue time on its queue, which otherwise serializes input delivery.
  * A burst of dummy matmuls on zeroed scratch runs during the initial
    DMA window so the PE HAM clock-gate un-throttles (1.2 -> 2.4 GHz)
    before real matmuls start.
  * Per-sample software pipeline: sample 0's attention+output overlaps
    sample 1's projections; output DMAs overlap compute.
"""

from contextlib import ExitStack

import numpy as np

import concourse.bacc as bacc
import concourse.mybir as mybir
from concourse import tile
from concourse.bass_utils import run_bass_kernel_spmd

F32 = mybir.dt.float32
F16 = mybir.dt.float16
AF = mybir.ActivationFunctionType
ALU = mybir.AluOpType
AX = mybir.AxisListType

NCORES = 8
B, C, CI, N = 16, 256, 128, 4096
BPC = B // NCORES  # samples per core
NQ = N // 128  # 32 column blocks of 128
EPS = 1e-5
NWARM = 8  # HAM warm-up matmuls (N=512 each, cold ~610ns)
DEDUPE_LDW = True

# const tensor column layout: [bt4(640) | wtp(512) | wr(640)]
CST_BT4 = 0
CST_WTP = 640
CST_WR = 1152
CST_COLS = 1792


def _build_nc():
    nc = bacc.Bacc("TRN2", target_bir_lowering=False, debug=False, num_devices=NCORES)

    x_d = nc.dram_tensor("x", [BPC, C, N], F16, kind="ExternalInput")
    cst_d = nc.dram_tensor("cst", [128, CST_COLS], F16, kind="ExternalInput")
    bg_d = nc.dram_tensor("bg", [CI, 1], F32, kind="ExternalInput")
    out_d = nc.dram_tensor("out", [BPC, C, N], F16, kind="ExternalOutput")

    with tile.TileContext(nc) as tc, ExitStack() as ctx:
        wpool = ctx.enter_context(tc.tile_pool(name="wts", bufs=1))
        xpool = ctx.enter_context(tc.tile_pool(name="xp", bufs=BPC))
        tppool = ctx.enter_context(tc.tile_pool(name="tpp", bufs=BPC))
        gpool = ctx.enter_context(tc.tile_pool(name="gp", bufs=BPC))
        zpool = ctx.enter_context(tc.tile_pool(name="zp", bufs=BPC))
        smpool = ctx.enter_context(tc.tile_pool(name="sm", bufs=BPC))
        opool = ctx.enter_context(tc.tile_pool(name="ost", bufs=4))
        ps_big = ctx.enter_context(tc.tile_pool(name="ps_big", bufs=3, space="PSUM"))
        ps_sml = ctx.enter_context(tc.tile_pool(name="ps_sml", bufs=2, space="PSUM"))

        # --- constants + scratch ---
        cst = wpool.tile([128, CST_COLS], F16, tag="cst")
        bg_sb = wpool.tile([CI, 1], F32, tag="bg")
        scr = wpool.tile([128, 512], F16, tag="scr")

        # warm-up: dummy matmuls during the input-DMA window so the HAM
        # clock gate reaches 8/8 before the first real matmul; cold N=512
        # matmuls (~610ns each) bridge until the first x chunks land.
        nc.vector.memset(scr[:, :], 0.0)
        ps_w = ps_big.tile([128, 1024], F32, tag="big", name="ps_warm")
        for _ in range(NWARM):
            nc.tensor.matmul(
                ps_w[:, 0:512], scr[:, 0:128], scr[:, :],
                start=True, stop=True, skip_group_check=True,
            )

        # --- input DMAs ---
        # x chunks alternate between the two HWDGE queues (Sync and ACT) so
        # the ~640ns-per-issue serialization doesn't throttle the feed.
        # Sample 1's chunks are issued mid-program from the ACT queue.
        xs = {}
        for b in range(BPC):
            xs[b] = xpool.tile([128, 2, N], F16, tag="x", name=f"x_{b}")

        def x_dma(eng, b, j):
            # 512-col chunk j of sample b
            eng.dma_start(
                xs[b][:, :, 512 * j : 512 * (j + 1)],
                x_d[b, :, 512 * j : 512 * (j + 1)].rearrange("(k p) n -> p k n", k=2),
            )

        nc.sync.dma_start(cst[:, :], cst_d[:])
        nc.sync.dma_start(xs[0][:, 0, 0:512], x_d[0, 0:128, 0:512])
        nc.scalar.dma_start(xs[0][:, 1, 0:512], x_d[0, 128:256, 0:512])
        for j in range(1, 8):
            x_dma(nc.sync if j % 2 else nc.scalar, 0, j)
        nc.scalar.dma_start(bg_sb[:, 0:1], bg_d[:])
        # sample-1 chunks queue on Sync behind sample 0's odd chunks; they
        # transfer ~11-19us, well before sample-1 compute needs them
        for m in range(4):
            nc.sync.dma_start(
                xs[1][:, :, 1024 * m : 1024 * (m + 1)],
                x_d[1, :, 1024 * m : 1024 * (m + 1)].rearrange("(k p) n -> p k n", k=2),
            )
        busy = {"act": 5 * 650.0, "dve": 0.0}

        wtp = cst[:, CST_WTP : CST_WTP + 512]
        w_g = cst[:, CST_WR : CST_WR + 256]
        w_w = cst[:, CST_WR + 256 : CST_WR + 512]
        idf = cst[:, CST_WR + 512 : CST_WR + 640]
        # bias row + ones row replicated at partitions 0/32/64/96 so up to 4
        # rank-1 bias matmuls can run in disjoint PE row groups concurrently
        btp = [cst[32 * r : 32 * r + 1, CST_BT4 : CST_BT4 + 512] for r in range(4)]
        onesr = [
            cst[32 * r : 32 * r + 1, CST_BT4 + 512 : CST_BT4 + 640] for r in range(4)
        ]

        # --- evacuation-engine load balancer (only ACT/DVE read PSUM) ---
        def cost(e, nlane):
            # ns per op with nlane elems per partition lane
            if e == "act":
                return (nlane + 352) / 1.2
            return nlane * 1.042 + 170.0

        def pick(nlane, allowed):
            e = min(allowed, key=lambda k: busy[k] + cost(k, nlane))
            busy[e] += cost(e, nlane)
            return e

        def evac_relu(dst, src, nlane, allowed=("act", "dve")):
            e = pick(nlane, allowed)
            if e == "act":
                nc.scalar.activation(dst, src, AF.Relu)
            else:
                nc.vector.tensor_scalar(dst, src, 0.0, None, ALU.max)

        def evac_relu_bias(dst, src, bias, nlane, allowed=("act", "dve")):
            e = pick(nlane, allowed)
            if e == "act":
                nc.scalar.activation(dst, src, AF.Relu, bias=bias)
            else:
                nc.vector.tensor_scalar(dst, src, bias, 0.0, ALU.add, ALU.max)

        def evac_copy(dst, src, nlane, allowed=("act", "dve")):
            e = pick(nlane, allowed)
            if e == "act":
                nc.scalar.copy(dst, src)
            else:
                nc.vector.tensor_copy(dst, src)

        tp_sb, g_sb, z_sb, f_soft, fT_sb = {}, {}, {}, {}, {}
        for b in range(BPC):
            tp_sb[b] = tppool.tile([128, 2 * N], F16, tag="tp", name=f"tp_{b}")
            g_sb[b] = gpool.tile([128, N], F16, tag="g", name=f"g_{b}")
            z_sb[b] = zpool.tile([128, N], F16, tag="z", name=f"z_{b}")

        def _thph_bias(pt, rbase):
            # 2 rank-1 bias matmuls (one per 512-col bank) in disjoint PE
            # row groups -> run concurrently with each other.
            for half in range(2):
                r = rbase + half
                nc.tensor.matmul(
                    pt[:, 512 * half : 512 * (half + 1)],
                    onesr[r], btp[r],
                    start=True, stop=False, skip_group_check=True,
                    tile_position=(32 * r, 0),
                )

        def _thph_body(b, pt, t, stop):
            for h in range(4):
                q = 4 * t + h
                for k in range(2):
                    nc.tensor.matmul(
                        pt[:, 256 * h : 256 * (h + 1)],
                        xs[b][:, k, 128 * q : 128 * (q + 1)],
                        wtp[:, 256 * k : 256 * (k + 1)],
                        start=False,
                        stop=(stop and h == 3 and k == 1),
                        skip_group_check=True,
                    )

        def _thph_evac(b, pt, t, allowed=("act", "dve")):
            # single contiguous 1024-wide evac: PSUM tile layout is already
            # the interleaved (th|ph per 256-block) SBUF layout
            evac_relu(
                tp_sb[b][:, 1024 * t : 1024 * (t + 1)], pt[:, :], 1024, allowed
            )

        def proj_thph(b, trange, allowed=("act", "dve")):
            for t in trange:
                pt = ps_big.tile([128, 1024], F32, tag="big", name=f"pt_{b}_{t}")
                _thph_bias(pt, 0)
                _thph_body(b, pt, t, True)
                _thph_evac(b, pt, t, allowed)

        def proj_g(b, trange=range(4), allowed=("act", "dve")):
            # evacuate g directly into the permuted layout
            # gP[d, 128q + c] = g[d, 32c + q]  (c = 16j + a, n = 32a + q)
            # single 4D-AP evac per (128,1024) PSUM tile.  k-outer matmul
            # order repeats the stationary operand so the redundant
            # LDWEIGHTS dedupes away.
            gPv = g_sb[b][:, :].rearrange("p (r t h a) -> p r t h a", r=NQ, t=4, h=2)
            for t in trange:
                pg = ps_big.tile([128, 1024], F32, tag="big", name=f"pg_{b}_{t}")
                for k in range(2):
                    for half in range(2):
                        j = 2 * t + half
                        nc.tensor.matmul(
                            pg[:, 512 * half : 512 * (half + 1)],
                            w_g[:, 128 * k : 128 * (k + 1)],
                            xs[b][:, k, 512 * j : 512 * (j + 1)],
                            start=(k == 0),
                            stop=(k == 1),
                            skip_group_check=True,
                        )
                src = pg[:, :].rearrange("p (h a r) -> p r h a", r=NQ, h=2)
                evac_relu_bias(
                    gPv[:, :, t, :, :], src, bg_sb[:, 0:1], 1024, allowed
                )

        def scores_part(b, qlo, qhi):
            # incremental slice of the (128,128) scores accumulation; the
            # partials interleave with projection matmuls so the PE stays
            # busy while x chunks land
            if qlo == 0:
                f_soft[b] = ps_sml.tile([128, 128], F32, tag="sml", name=f"ps_s_{b}")
            ps_s = f_soft[b]
            for q in range(qlo, qhi):
                nc.tensor.matmul(
                    ps_s[:, :],
                    tp_sb[b][:, 256 * q : 256 * q + 128],
                    tp_sb[b][:, 256 * q + 128 : 256 * (q + 1)],
                    start=(q == 0),
                    stop=(q == NQ - 1),
                    skip_group_check=True,
                )

        def softmax_ops(b):
            ps_s = f_soft[b]
            negmax = smpool.tile([128, 1], F32, tag="negmax", name=f"negmax_{b}")
            nc.vector.reduce_max(negmax[:, :], ps_s[:, :], axis=AX.X, negate=True)
            e_sb = smpool.tile([128, 128], F16, tag="e_sb", name=f"e_sb_{b}")
            sumex = smpool.tile([128, 1], F32, tag="sumex", name=f"sumex_{b}")
            nc.scalar.activation(
                e_sb[:, :], ps_s[:, :], AF.Exp, bias=negmax[:, :], accum_out=sumex[:, :]
            )
            rs = smpool.tile([128, 1], F32, tag="rs", name=f"rs_{b}")
            nc.vector.reciprocal(rs[:, :], sumex[:, :])
            f_sb = smpool.tile([128, 128], F16, tag="f_sb", name=f"f_sb_{b}")
            nc.scalar.activation(f_sb[:, :], e_sb[:, :], AF.Copy, scale=rs[:, :])
            f_soft[b] = f_sb
            busy["act"] += cost("act", 128) + cost("act", 128)
            busy["dve"] += 300.0

        def transpose_f(b):
            ps_t = ps_sml.tile([128, 128], F16, tag="sml", name=f"ps_t_{b}")
            nc.tensor.transpose(ps_t[:, :], f_soft[b][:, :], idf[:, :])
            fT = smpool.tile([128, 128], F16, tag="fT", name=f"fT_{b}")
            nc.vector.tensor_copy(fT[:, :], ps_t[:, :])
            busy["dve"] += cost("dve", 128)
            fT_sb[b] = fT

        def zblk(b, t):
            # z block t: 8 permuted-g matmuls + one 1024-wide PSUM->SBUF copy
            pz = ps_big.tile([128, 1024], F32, tag="big", name=f"pz_{b}_{t}")
            for tq in range(8):
                q = 8 * t + tq
                nc.tensor.matmul(
                    pz[:, 128 * tq : 128 * (tq + 1)],
                    g_sb[b][:, 128 * q : 128 * (q + 1)],
                    fT_sb[b][:, :],
                    start=True,
                    stop=True,
                    skip_group_check=True,
                )
            evac_copy(z_sb[b][:, 1024 * t : 1024 * (t + 1)], pz[:, :], 1024)

        def fin_pair(b, t, split_out=False, allowed=("dve",)):
            # final projection + residual for n-chunks 2t and 2t+1 (each 512
            # cols, both 128-row output halves) -> two (128,1024) PSUM
            # tiles, one evac op each.  Matmuls are grouped per stationary
            # (w_w half / idf) so redundant LDWEIGHTS dedupe away.
            z = z_sb[b]
            ob = opool.tile([128, 2, 1024], F16, tag="ost", name=f"ot_{b}_{t}")
            pfs, engs = [], []
            for p in range(2):
                pfs.append(
                    ps_big.tile([128, 1024], F32, tag="big", name=f"pf_{b}_{t}_{p}")
                )
                engs.append(pick(1024, allowed))
            if "act" in engs:
                # residual via identity matmul for the ACT-evacuated tiles
                for p in range(2):
                    if engs[p] != "act":
                        continue
                    for h in range(2):
                        nc.tensor.matmul(
                            pfs[p][:, 512 * h : 512 * (h + 1)],
                            idf, xs[b][:, h, 512 * (2 * t + p) : 512 * (2 * t + p + 1)],
                            start=True, stop=False, skip_group_check=True,
                        )
            for h in range(2):
                for p in range(2):
                    jj = 2 * t + p
                    nc.tensor.matmul(
                        pfs[p][:, 512 * h : 512 * (h + 1)],
                        w_w[:, 128 * h : 128 * (h + 1)],
                        z[:, 512 * jj : 512 * (jj + 1)],
                        start=(engs[p] != "act"),
                        stop=True, skip_group_check=True,
                    )
            for p in range(2):
                jj = 2 * t + p
                ot = ob[:, :, 512 * p : 512 * (p + 1)]
                pfv = pfs[p][:, :].rearrange("p (h n) -> p h n", h=2)
                if engs[p] == "act":
                    nc.scalar.copy(ot, pfv)
                else:
                    # residual fused into the DVE evacuation
                    nc.vector.scalar_tensor_tensor(
                        ot, pfv, 0.0,
                        xs[b][:, :, 512 * jj : 512 * (jj + 1)],
                        ALU.add, ALU.add,
                    )
                if split_out:
                    nc.sync.dma_start(
                        out_d[
                            b, :, 1024 * t + 512 * p : 1024 * t + 512 * (p + 1)
                        ].rearrange("(h p) n -> p h n", h=2),
                        ot,
                    )
            if not split_out:
                nc.sync.dma_start(
                    out_d[b, :, 1024 * t : 1024 * (t + 1)].rearrange(
                        "(h p) n -> p h n", h=2
                    ),
                    ob[:, :, :],
                )

        # --- software pipeline over the 2 samples ---
        # Loop 1 (sample 0, DMA-feed-limited window): consume each landed
        # 1024-col x chunk completely - 2 thph tiles + 1 g tile + the
        # previous chunk's scores partials - so the PE never idles long
        # enough for the HAM clock gate to re-throttle.  The first three
        # thph evacs go to DVE only (ACT is still busy with DMA issues and
        # its activation-table load).
        for c in range(4):
            proj_thph(
                0, (2 * c, 2 * c + 1),
                allowed=("dve",) if c == 0 else ("act", "dve"),
            )
            proj_g(0, (c,))
            if c > 0:
                scores_part(0, 8 * (c - 1), 8 * c)
        scores_part(0, 24, 32)
        softmax_ops(0)
        # Loop 2 (sample 1 projections + sample 0 attention/output): the
        # x1 feed window gets sample 0's z/fin PE work interleaved so both
        # PE and the evacuation engines see a smooth load.
        proj_thph(1, (0,))
        transpose_f(0)
        proj_thph(1, (1,))
        for c in range(4):
            if c > 0:
                proj_thph(1, (2 * c, 2 * c + 1))
            if c < 3:
                zblk(0, c)
            proj_g(1, (c,))
            if c > 0:
                scores_part(1, 8 * (c - 1), 8 * c)
            if c < 3:
                fin_pair(0, c)
        scores_part(1, 24, 32)
        # sample-0's last attention block covers sample-1's softmax latency
        zblk(0, 3)
        softmax_ops(1)
        transpose_f(1)
        fin_pair(0, 3)
        # tail: z blocks run one step ahead of their finals so fin matmuls
        # never wait on the immediately-preceding z evacuation
        zblk(1, 0)
        zblk(1, 1)
        fin_pair(1, 0, allowed=("act", "dve"))
        zblk(1, 2)
        fin_pair(1, 1, allowed=("act", "dve"))
        zblk(1, 3)
        fin_pair(1, 2, allowed=("act", "dve"))
        fin_pair(1, 3, split_out=True, allowed=("act", "dve"))

    if DEDUPE_LDW:
        _dedupe_ldweights(nc)
    nc.compile()
    return nc


def _dedupe_ldweights(nc):
    """Remove back-to-back InstLdweights with identical weight APs.

    tile_legalize emits an Ldweights before every Matmult even when the
    stationary operand is unchanged; the redundant load costs ~45ns of
    exposed PE time per N=512 matmul.  The PE array keeps the stationary
    operand until the next Ldweights, so dropping an identical reload (with
    only Matmults in between) is semantics-preserving.  Any dependency
    edges on the dropped instruction are merged into the following PE
    instruction so semaphore generation still sees them.
    """
    for f in nc.m.functions:
        for blk in f.blocks:
            insts = blk.instructions
            keep = []
            last_sig = None
            pending = None  # removed ldweights whose deps need a new home
            for inst in insts:
                if getattr(inst, "engine", None) != mybir.EngineType.PE:
                    keep.append(inst)
                    continue
                if isinstance(inst, mybir.InstLdweights):
                    sig = (
                        str(inst.ins[0]),
                        str(inst.perf_mode),
                        str(inst.is_transpose),
                    )
                    if sig == last_sig:
                        pending = inst
                        continue
                    last_sig = sig
                elif not isinstance(inst, mybir.InstMatmult):
                    # branches / drains / semaphores invalidate the cache
                    last_sig = None
                if pending is not None:
                    inst.merge_dependencies_from(pending)
                    pending = None
                keep.append(inst)
            assert pending is None
            if len(keep) != len(insts):
                blk.instructions = keep


_CACHE = {}


def _prepare(inputs):
    """Fold BN into weights/biases and build per-core input maps."""

    def fold(w, bias, gamma, beta, mean, var):
        inv = gamma / np.sqrt(var + EPS)
        return (w * inv[:, None]).astype(np.float32), (
            beta + (bias - mean) * inv
        ).astype(np.float32)

    Wg, bg = fold(
        inputs["g_w"], inputs["g_b"], inputs["g_gamma"], inputs["g_beta"],
        inputs["g_mean"], inputs["g_var"],
    )
    Wth, bth = fold(
        inputs["th_w"], inputs["th_b"], inputs["th_gamma"], inputs["th_beta"],
        inputs["th_mean"], inputs["th_var"],
    )
    Wph, bph = fold(
        inputs["ph_w"], inputs["ph_b"], inputs["ph_gamma"], inputs["ph_beta"],
        inputs["ph_mean"], inputs["ph_var"],
    )
    Ww, bw = fold(
        inputs["w_w"], inputs["w_b"], inputs["w_gamma"], inputs["w_beta"],
        inputs["w_mean"], inputs["w_var"],
    )

    # x_adj = x + bw (per out-channel); compensate projection biases.
    x = np.asarray(inputs["x"], dtype=np.float32).reshape(B, C, N)
    x_adj = (x + bw[None, :, None]).astype(np.float16)
    bg_a = bg - Wg @ bw
    bth_a = bth - Wth @ bw
    bph_a = bph - Wph @ bw

    WgT = np.ascontiguousarray(Wg.T)  # (256, 128)
    wg_host = np.concatenate([WgT[0:128], WgT[128:256]], axis=1)  # (128, 256)
    WtpT = np.concatenate([Wth.T, Wph.T], axis=1)  # (256, 256)
    wtp_host = np.concatenate([WtpT[0:128], WtpT[128:256]], axis=1)  # (128, 512)
    btp_host = np.concatenate([bth_a, bph_a, bth_a, bph_a])  # (512,)
    ww_host = np.ascontiguousarray(Ww.T)  # (128, 256)

    bt4 = np.zeros((128, 640), dtype=np.float16)
    for r in (0, 32, 64, 96):
        bt4[r, 0:512] = btp_host
        bt4[r, 512:640] = 1.0
    wr = np.zeros((128, 640), dtype=np.float16)
    wr[:, 0:256] = wg_host
    wr[:, 256:512] = ww_host
    wr[:, 512:640] = np.eye(128, dtype=np.float16)
    cst = np.concatenate([bt4, wtp_host.astype(np.float16), wr], axis=1)
    consts = {
        "cst": np.ascontiguousarray(cst, dtype=np.float16),
        "bg": np.ascontiguousarray(bg_a.reshape(CI, 1), dtype=np.float32),
    }
    in_maps = []
    for i in range(NCORES):
        m = dict(consts)
        m["x"] = np.ascontiguousarray(x_adj[BPC * i : BPC * (i + 1)])
        in_maps.append(m)
    return in_maps


def _get_nc():
    if "nc" not in _CACHE:
        _CACHE["nc"] = _build_nc()
    return _CACHE["nc"]


def run(inputs, **kw):
    """Run on hardware; returns (full_output, BassKernelResults)."""
    nc = _get_nc()
    in_maps = _prepare(inputs)
    res = run_bass_kernel_spmd(nc, in_maps, list(range(NCORES)), **kw)
    out = np.concatenate(
        [
            np.asarray(res.results[i]["out"], dtype=np.float32).reshape(BPC, C, 64, 64)
            for i in range(NCORES)
        ],
        axis=0,
    )
    return np.ascontiguousarray(out), res


def kernel(**inputs):
    out, _ = run(inputs)
    return out
